# revision 2
# baseline (speedup 1.0000x reference)
"""Trainium2 kernel for nn_MessageFunction: wide-PSUM groups + split drain.

HW-measured design (trn2, slope-timed):
- PE schedule: per (half, m) group hold each stationary weight tile for 4
  consecutive 512-col matmuls (stationary switches cost ~190ns each when
  per-matmul; 4x reuse amortizes), rotating across the group's 4 PSUM banks
  inner-loop (back-to-back accumulation into one bank stalls the PE), two
  4-bank groups alternating via pool bufs=2.
- One [128, 2048] PSUM tile per group (4 whole banks): matmuls write 512-col
  bank slices; drain is TWO wide instructions instead of four narrow ones —
  ACT does cols 0:1024 (+bias), DVE does 1024:2048 (+bias via
  tensor_scalar_add). Splitting the drain across both engines measured 2x
  faster than either engine alone (137us -> 67us pass, zero-data regime).
- Inputs prefetched one half-pass ahead (explicit double buffers) so input
  transfers never queue behind output DMAs on the rings.
- fp16 output, host upcasts to fp32 (err ~4e-4 total incl fp16 matmuls).
- Rings: e + even-m outs on sync(SP), h + odd-m outs on scalar(ACT).
"""
import numpy as np
import concourse.tile as tile
from concourse import bacc, mybir
from concourse.bass_utils import run_bass_kernel_spmd

try:
    import jax
    jax.config.update("jax_compilation_cache_dir", "/tmp/.jax_kernel_cache")
    jax.config.update("jax_persistent_cache_min_compile_time_secs", 0.5)
except Exception:
    pass

B, D, NN = 128, 768, 256
NCORES = 8
BPC = B // NCORES          # 16 batches per core
PAIR = 2                   # batches per 512-wide sub-block
NBLK = BPC // PAIR         # 8 sub-blocks per pass
NCOL = PAIR * NN           # 512 moving columns per matmul
HALF = 4                   # sub-blocks per half
HCOL = HALF * NCOL         # 2048 columns per half
KT = D // 128              # 6 contraction tiles per input matrix
MT = D // 128              # 6 output row tiles
F32 = mybir.dt.float32
DT = mybir.dt.float16
NPDT = np.float16


def build(repeat: int = 1, loop_repeat: int = 1):
    nc = bacc.Bacc("TRN2", target_bir_lowering=False, debug=False,
                   num_devices=NCORES)
    e = nc.dram_tensor("e", [KT, 128, BPC * NN], DT, kind="ExternalInput").ap()
    h = nc.dram_tensor("h", [KT, 128, BPC * NN], DT, kind="ExternalInput").ap()
    weT = nc.dram_tensor("weT", [D, D], DT, kind="ExternalInput").ap()
    wwT = nc.dram_tensor("wwT", [D, D], DT, kind="ExternalInput").ap()
    bias = nc.dram_tensor("bias", [D], F32, kind="ExternalInput").ap()
    out = nc.dram_tensor("out", [BPC, D, NN], DT, kind="ExternalOutput").ap()

    weT_v = weT.rearrange("(k p) (m q) -> p k m q", p=128, q=128)
    wwT_v = wwT.rearrange("(k p) (m q) -> p k m q", p=128, q=128)
    bias_v = bias.rearrange("(m p) -> p m", p=128)          # [128,6]
    out_v = out.rearrange("b (m p) n -> p m b n", p=128)    # [128,6,16,256]

    with tile.TileContext(nc) as tc:
        with (
            tc.tile_pool(name="wpool", bufs=1) as wpool,
            tc.tile_pool(name="opool", bufs=4) as opool,
            # one [128, 2048] tile (4 banks) per group x bufs=2 = 8 banks
            tc.tile_pool(name="pspool", bufs=2, space="PSUM") as pspool,
        ):
            we_t = wpool.tile([128, KT, MT, 128], DT)
            ww_t = wpool.tile([128, KT, MT, 128], DT)
            bias_t = wpool.tile([128, MT], F32)
            ebuf = [wpool.tile([128, KT, HCOL], DT, name=f"eb{i}")
                    for i in range(2)]
            hbuf = [wpool.tile([128, KT, HCOL], DT, name=f"hb{i}")
                    for i in range(2)]

            nc.sync.dma_start(bias_t[:], bias_v)
            nc.sync.dma_start(we_t[:], weT_v)
            nc.scalar.dma_start(ww_t[:], wwT_v)

            def load(buf, half):
                cs = slice(half * HCOL, (half + 1) * HCOL)
                for k in range(KT):
                    nc.sync.dma_start(ebuf[buf][:, k], e[k, :, cs])
                    nc.scalar.dma_start(hbuf[buf][:, k], h[k, :, cs])

            def compute(buf, half):
                et, ht = ebuf[buf], hbuf[buf]
                for m in range(MT):
                    ps = pspool.tile([128, HCOL], F32, name="ps")
                    for k in range(KT):
                        for c4 in range(HALF):
                            nc.tensor.matmul(
                                ps[:, c4 * NCOL:(c4 + 1) * NCOL],
                                we_t[:, k, m, :],
                                et[:, k, c4 * NCOL:(c4 + 1) * NCOL],
                                start=(k == 0), stop=False)
                    for k in range(KT):
                        for c4 in range(HALF):
                            nc.tensor.matmul(
                                ps[:, c4 * NCOL:(c4 + 1) * NCOL],
                                ww_t[:, k, m, :],
                                ht[:, k, c4 * NCOL:(c4 + 1) * NCOL],
                                start=False, stop=(k == KT - 1))
                    res = opool.tile([128, HCOL], DT, name="res")
                    HW2 = HCOL // 2
                    nc.scalar.activation(
                        res[:, :HW2], ps[:, :HW2],
                        mybir.ActivationFunctionType.Identity,
                        bias=bias_t[:, m:m + 1], scale=1.0)
                    nc.vector.tensor_scalar_add(
                        res[:, HW2:], ps[:, HW2:], bias_t[:, m:m + 1])
                    oq = nc.sync if m % 2 == 0 else nc.scalar
                    oq.dma_start(
                        out_v[:, m, half * 2 * HALF:(half + 1) * 2 * HALF, :],
                        res[:].rearrange("p (b n) -> p b n", b=2 * HALF))

            load(0, 0)  # prologue

            def body():
                for _ in range(repeat):
                    load(1, 1)      # prefetch half1 while computing half0
                    compute(0, 0)
                    load(0, 0)      # prefetch next iteration's half0
                    compute(1, 1)

            if loop_repeat > 1:
                with tc.For_i(0, loop_repeat, 1,
                              hint_engines=(mybir.EngineType.PE,)):
                    body()
            else:
                body()
    nc.compile()
    return nc


def _prep_in_maps(h_w, e_vw, We, be, Ww, bw):
    e_vw = np.asarray(e_vw, dtype=np.float32).astype(NPDT)
    h_w = np.asarray(h_w, dtype=np.float32).astype(NPDT)
    weT = np.ascontiguousarray(np.asarray(We, dtype=np.float32).T).astype(NPDT)
    wwT = np.ascontiguousarray(np.asarray(Ww, dtype=np.float32).T).astype(NPDT)
    bias = (np.asarray(be, dtype=np.float32)
            + np.asarray(bw, dtype=np.float32)).astype(np.float32)

    def slab(x, c):
        # [BPC, D, NN] -> [KT, 128, BPC*NN] : slab[k, p, b*NN+n] = x[b, k*128+p, n]
        s = x[c * BPC:(c + 1) * BPC].reshape(BPC, KT, 128, NN)
        return np.ascontiguousarray(s.transpose(1, 2, 0, 3).reshape(KT, 128, BPC * NN))

    return [
        {"e": slab(e_vw, c), "h": slab(h_w, c),
         "weT": weT, "wwT": wwT, "bias": bias}
        for c in range(NCORES)
    ]


_NC_CACHE = []


def kernel(h_v, h_w, e_vw, We, be, Ww, bw):
    if not _NC_CACHE:
        _NC_CACHE.append(build())
    nc = _NC_CACHE[0]
    in_maps = _prep_in_maps(h_w, e_vw, We, be, Ww, bw)
    r = run_bass_kernel_spmd(nc, in_maps, core_ids=list(range(NCORES)))
    return np.concatenate(
        [r.results[c]["out"] for c in range(NCORES)], axis=0).astype(np.float32)
